# revision 7
# baseline (speedup 1.0000x reference)
"""Focal-loss (2-class cross-entropy) sum on 8 TRN2 NeuronCores.

Data-parallel: pred [16777216, 2] f32 and gold [16777216] f32 are split
along the batch axis into 8 equal shards; each core computes partial
sums; the host combines the 8 partials into the final scalar.

Math (per row, d = p1 - p0, t = gold >= 0.5):
    sp  = softplus(d)  = -log p0        spn = softplus(-d) = -log p1
    s2  = sigmoid(d)^2 = exp(-2*spn)    u2  = sigmoid(-d)^2 = exp(-2*sp)
    loss = (0.75 - 0.1875 t) * sp * s2 + 0.25 t * spn * u2
         = 4*X + t*(Y - X)
    where X = 0.1875 * sp * s2, Y = 0.25 * spn * u2.
All transcendentals use the Exp/Ln pair (one ACT table set):
    E = exp(d); sp = ln(E + 1); spn = sp - d
    s2' = exp(-2*spn + ln 0.1875); u2' = exp(-2*sp + ln 0.25)
Per-core output: out[128, 2*NT] holding per-partition partial sums of X
(cols 0:NT) and t*(Y-X) (cols NT:2NT); host reduces in float64.
"""

import math

import numpy as np

import concourse.bass as bass
import concourse.tile as tile
from concourse import bacc, mybir
from concourse.bass_utils import run_bass_kernel_spmd

AF = mybir.ActivationFunctionType
OP = mybir.AluOpType
F32 = mybir.dt.float32

N = 16777216
NCORES = 8
R = N // NCORES  # rows per core
P = 128  # SBUF partitions
F = 2048  # rows per partition per tile
NT = R // (P * F)  # tiles per core

LN_X = math.log(0.1875)  # fold 0.1875 into s2's exp bias
LN_Y = math.log(0.25)  # fold 0.25 into u2's exp bias


def build_program(rows: int = R, f: int = F, reps: int = 1):
    """reps>1 repeats the whole compute loop (same data) for slope timing."""
    nt = rows // (P * f)
    assert nt * P * f == rows
    nc = bacc.Bacc(
        "TRN2", target_bir_lowering=False, debug=False, num_devices=NCORES
    )
    # Const APs for the activation bias immediates (framework pre-registers
    # only 0.0/1.0).
    for value in (LN_X, LN_Y):
        t = nc.alloc_sbuf_tensor(f"const-float32-{value}", [128, 1], F32)
        nc.gpsimd.memset(t.ap(), value)
        nc.const_aps.aps[(F32, value)] = t.ap()
    nc.all_engine_barrier()
    pred = nc.dram_tensor("pred", [rows, 2], F32, kind="ExternalInput").ap()
    gold = nc.dram_tensor("gold", [rows], F32, kind="ExternalInput").ap()
    out = nc.dram_tensor("out", [P, 2 * nt], F32, kind="ExternalOutput").ap()

    pred_r = pred.rearrange("(n p f) c -> n p (f c)", p=P, f=f)  # [nt,128,2f]
    gold_r = gold.rearrange("(n p f) -> n p f", p=P, f=f)  # [nt,128,f]

    with tile.TileContext(nc) as tc:
        with (
            tc.tile_pool(name="io", bufs=3) as io_pool,
            tc.tile_pool(name="work", bufs=2) as work,
            tc.tile_pool(name="acc", bufs=1) as accp,
        ):
            acc_x = accp.tile([P, nt], F32)
            acc_g = accp.tile([P, nt], F32)
            for i in range(nt * reps):
                i = i % nt
                pt = io_pool.tile([P, 2 * f], F32, tag="pred")
                nc.sync.dma_start(pt[:], pred_r[i])
                gt = io_pool.tile([P, f], F32, tag="gold")
                nc.sync.dma_start(gt[:], gold_r[i])

                pv = pt[:].rearrange("p (f c) -> p f c", c=2)
                d = work.tile([P, f], F32, tag="d_Y")
                nc.vector.tensor_sub(d[:], pv[:, :, 1], pv[:, :, 0])

                e = work.tile([P, f], F32, tag="E_X")
                nc.scalar.activation(e[:], d[:], AF.Exp)
                sp = work.tile([P, f], F32, tag="sp")
                nc.scalar.activation(sp[:], e[:], AF.Ln, bias=1.0)
                spn = work.tile([P, f], F32, tag="spn")
                nc.vector.scalar_tensor_tensor(
                    spn[:], d[:], -1.0, sp[:], op0=OP.mult, op1=OP.add
                )
                s2 = work.tile([P, f], F32, tag="s2_G")
                nc.scalar.activation(s2[:], spn[:], AF.Exp, bias=LN_X, scale=-2.0)
                u2 = work.tile([P, f], F32, tag="u2_tG")
                nc.scalar.activation(u2[:], sp[:], AF.Exp, bias=LN_Y, scale=-2.0)

                # X = sp * s2' (= 0.1875*sp*sigmoid(d)^2), with fused row sum
                # (tensor_tensor_reduce crashes this runtime's exec unit, so
                # the multiply rides a scalar_tensor_tensor with accum_out)
                x = work.tile([P, f], F32, tag="E_X")
                nc.vector.scalar_tensor_tensor(
                    x[:],
                    sp[:],
                    1.0,
                    s2[:],
                    op0=OP.mult,
                    op1=OP.mult,
                    accum_out=acc_x[:, i : i + 1],
                )
                # Y = spn * u2' (= 0.25*spn*sigmoid(-d)^2)
                y = work.tile([P, f], F32, tag="d_Y")
                nc.vector.tensor_mul(y[:], spn[:], u2[:])
                # G = Y - X
                g = work.tile([P, f], F32, tag="s2_G")
                nc.vector.scalar_tensor_tensor(
                    g[:], x[:], -1.0, y[:], op0=OP.mult, op1=OP.add
                )
                # t*G with fused row sum; t = (gold >= 0.5)
                tg = work.tile([P, f], F32, tag="u2_tG")
                nc.vector.scalar_tensor_tensor(
                    tg[:],
                    gt[:],
                    0.5,
                    g[:],
                    op0=OP.is_ge,
                    op1=OP.mult,
                    accum_out=acc_g[:, i : i + 1],
                )
            nc.sync.dma_start(out[:, :nt], acc_x[:])
            nc.sync.dma_start(out[:, nt:], acc_g[:])
    nc.compile()
    return nc


def build_program_v2(rows: int = R, f: int = F, reps: int = 1, kb: int = 8):
    """Two-phase variant: Softplus-set batch then Exp-set batch per KB tiles.

    Phase 1 (per tile): d = p1-p0; sp = softplus(d); spn = softplus(-d).
    Phase 2 (per tile): s2' = exp(-2 spn + ln .1875); u2' = exp(-2 sp + ln .25)
        X = sp*s2' (accum); Y = spn*u2'; tX = t*X (accum); tY = t*Y (accum).
    total = 4*accX - accTX + accTY. 5 DVE ops/tile vs 6 in v1; 2 ACT table
    sets per KB-tile batch instead of per-op thrash.
    """
    nt = rows // (P * f)
    assert nt * P * f == rows and nt % kb == 0
    nc = bacc.Bacc(
        "TRN2", target_bir_lowering=False, debug=False, num_devices=NCORES
    )
    for value in (LN_X, LN_Y):
        t = nc.alloc_sbuf_tensor(f"const-float32-{value}", [128, 1], F32)
        nc.gpsimd.memset(t.ap(), value)
        nc.const_aps.aps[(F32, value)] = t.ap()
    nc.all_engine_barrier()
    pred = nc.dram_tensor("pred", [rows, 2], F32, kind="ExternalInput").ap()
    gold = nc.dram_tensor("gold", [rows], F32, kind="ExternalInput").ap()
    out = nc.dram_tensor("out", [P, 3 * nt], F32, kind="ExternalOutput").ap()

    pred_r = pred.rearrange("(n p f) c -> n p (f c)", p=P, f=f)
    gold_r = gold.rearrange("(n p f) -> n p f", p=P, f=f)

    with tile.TileContext(nc) as tc:
        with (
            tc.tile_pool(name="io", bufs=3) as io_pool,
            tc.tile_pool(name="sps", bufs=2 * kb) as spp,
            tc.tile_pool(name="work", bufs=3) as work,
            tc.tile_pool(name="acc", bufs=1) as accp,
        ):
            acc_x = accp.tile([P, nt], F32)
            acc_tx = accp.tile([P, nt], F32)
            acc_ty = accp.tile([P, nt], F32)
            for ib in range((nt * reps) // kb):
                sps = []
                for j in range(kb):
                    i = (ib * kb + j) % nt
                    pt = io_pool.tile([P, 2 * f], F32, tag="pred")
                    nc.sync.dma_start(pt[:], pred_r[i])
                    pv = pt[:].rearrange("p (f c) -> p f c", c=2)
                    d = work.tile([P, f], F32, tag="d_Y")
                    nc.vector.tensor_sub(d[:], pv[:, :, 1], pv[:, :, 0])
                    sp = spp.tile([P, f], F32, tag="sp")
                    nc.scalar.activation(sp[:], d[:], AF.Softplus)
                    spn = spp.tile([P, f], F32, tag="spn")
                    nc.scalar.activation(spn[:], d[:], AF.Softplus, scale=-1.0)
                    sps.append((i, sp, spn))
                for i, sp, spn in sps:
                    s2 = work.tile([P, f], F32, tag="s2_G")
                    nc.scalar.activation(s2[:], spn[:], AF.Exp, bias=LN_X, scale=-2.0)
                    u2 = work.tile([P, f], F32, tag="u2_tG")
                    nc.scalar.activation(u2[:], sp[:], AF.Exp, bias=LN_Y, scale=-2.0)
                    gt = io_pool.tile([P, f], F32, tag="gold")
                    nc.sync.dma_start(gt[:], gold_r[i])
                    x = work.tile([P, f], F32, tag="X")
                    nc.vector.scalar_tensor_tensor(
                        x[:], sp[:], 1.0, s2[:], op0=OP.mult, op1=OP.mult,
                        accum_out=acc_x[:, i : i + 1],
                    )
                    y = work.tile([P, f], F32, tag="d_Y")
                    nc.vector.tensor_mul(y[:], spn[:], u2[:])
                    tx = work.tile([P, f], F32, tag="tX")
                    nc.vector.scalar_tensor_tensor(
                        tx[:], gt[:], 0.5, x[:], op0=OP.is_ge, op1=OP.mult,
                        accum_out=acc_tx[:, i : i + 1],
                    )
                    ty = work.tile([P, f], F32, tag="tY")
                    nc.vector.scalar_tensor_tensor(
                        ty[:], gt[:], 0.5, y[:], op0=OP.is_ge, op1=OP.mult,
                        accum_out=acc_ty[:, i : i + 1],
                    )
            nc.sync.dma_start(out[:, :nt], acc_x[:])
            nc.sync.dma_start(out[:, nt : 2 * nt], acc_tx[:])
            nc.sync.dma_start(out[:, 2 * nt :], acc_ty[:])
    nc.compile()
    return nc


_CACHE: dict = {}


def kernel(pred: np.ndarray, gold: np.ndarray) -> np.ndarray:
    if "nc" not in _CACHE:
        _CACHE["nc"] = build_program()
    nc = _CACHE["nc"]

    pred = np.asarray(pred, dtype=np.float32).reshape(NCORES, R, 2)
    gold = np.asarray(gold, dtype=np.float32).reshape(NCORES, R)
    in_maps = [
        {"pred": np.ascontiguousarray(pred[i]), "gold": np.ascontiguousarray(gold[i])}
        for i in range(NCORES)
    ]
    res = run_bass_kernel_spmd(nc, in_maps, list(range(NCORES))).results
    total = np.float64(0.0)
    for r in res:
        o = np.asarray(r["out"], dtype=np.float64)
        total += 4.0 * o[:, :NT].sum() + o[:, NT:].sum()
    return np.array(np.float32(total))
